# revision 1
# baseline (speedup 1.0000x reference)
"""Trainium2 Bass kernel for nn_BRNN_8151847927833.

Binary RNN: B=64 seqs, T=512 steps, d_model=1024, 6 binary FF layers per
step, then a small head + log_softmax + NLL loss averaged over (t, b).

Strategy (data-parallel over batch, 8 cores x 8 sequences):
  - All weights are +-1 (sign of latents), thresholds are small integers.
    Matmuls are therefore EXACT in low precision: products are +-1 and
    PSUM accumulates in fp32.
  - Activations are kept transposed: x^T stored as [128 partitions, 64]
    where column = m_chunk*8 + b (8 chunks of 128 dims x 8 batch).
    Weight-stationary matmuls (lhsT = W chunk [128k x 128m], moving
    rhs = x^T chunk [128, 8]) produce the NEXT transposed layout
    directly -> zero transposes in the whole recurrence.
  - Activations use a {0,1} encoding (h' = (h+1)/2) so the per-layer
    nonlinearity is a single DVE is_ge against a host-folded threshold
    (thr + colsum(W))/2 — exact integer-vs-half-integer compare, and no
    ScalarE hop on the recurrence critical path.  The activation is
    split per 128-dim chunk so each chunk unblocks the next layer's
    matmuls as soon as its PSUM accumulation group completes.
  - The head + log-softmax + token-gather do NOT feed the recurrence, so
    they are deferred: the 128 "read" dims per step are stored to a
    [128, T*8] buffer and processed as 32 dense batched matmul tiles
    after the T-loop.  No max-subtraction needed: |logits| <= 8.
  - Each core returns per-partition partial sums of (logsumexp - logit_tok);
    the host sums across cores and divides by B*T.
"""

import math
import sys

import numpy as np

sys.path.insert(0, "/opt/trn_rl_repo")

import ml_dtypes  # noqa: E402

import concourse.bass as bass  # noqa: E402
import concourse.bacc as bacc  # noqa: E402
import concourse.mybir as mybir  # noqa: E402
from concourse.tile import TileContext  # noqa: E402
from concourse.bass_utils import run_bass_kernel_spmd  # noqa: E402

F32 = mybir.dt.float32
BF16 = mybir.dt.bfloat16
FP8 = mybir.dt.float8e4
NP_BF16 = ml_dtypes.bfloat16
NP_FP8 = ml_dtypes.float8_e4m3

D = 1024          # d_model
KC = 8            # contraction chunks of 128
MC = 8            # output chunks of 128
NUMFF = 6
VOCAB = 128
READ = 128
CARRY = 896
BL = 8            # batch per core
NCORES = 8
LOGIT_SCALE = 1.0 / 16.0

# weight dtype for the FF stack / head (fp8 -> fast weight load)
W_DT = FP8
W_NP = NP_FP8


def build_nc(T):
    """Build the SPMD Bass kernel for T timesteps (BL sequences/core)."""
    ntile = T * BL // 128  # tail tiles over (t, b)
    assert T * BL % 128 == 0

    nc = bacc.Bacc("TRN2", target_bir_lowering=False)
    wff = nc.dram_tensor("wff", [128, NUMFF * KC * D], W_DT, kind="ExternalInput")
    emb = nc.dram_tensor("emb", [128, T * BL], BF16, kind="ExternalInput")
    x0 = nc.dram_tensor("x0", [128, MC * BL], BF16, kind="ExternalInput")
    thr = nc.dram_tensor("thr", [128, NUMFF * MC * BL], F32, kind="ExternalInput")
    headw = nc.dram_tensor("headw", [128, VOCAB], W_DT, kind="ExternalInput")
    oneh = nc.dram_tensor("oneh", [128, ntile * VOCAB], F32, kind="ExternalInput")
    wexp = nc.dram_tensor("wexp", [128, 4 * VOCAB], F32, kind="ExternalInput")
    res = nc.dram_tensor("res", [128, 1], F32, kind="ExternalOutput")

    AT = mybir.ActivationFunctionType
    ALU = mybir.AluOpType

    with TileContext(nc) as tc:
        with (
            tc.tile_pool(name="const", bufs=1) as cpool,
            tc.tile_pool(name="work", bufs=4) as wpool,
            tc.tile_pool(name="hpool", bufs=3) as hpool,
            tc.tile_pool(name="pst", bufs=1, space="PSUM") as pstpool,
            tc.tile_pool(name="ps2pool", bufs=2, space="PSUM") as ps2pool,
        ):
            # ---- resident inputs (DMA in consumption order) ----
            xb = cpool.tile([128, MC * BL], BF16, tag="xb")
            nc.sync.dma_start(out=xb[:, :], in_=x0[:, :])
            thrsb = cpool.tile([128, NUMFF * MC * BL], F32, tag="thrsb")
            nc.sync.dma_start(out=thrsb[:, :], in_=thr[:, :])
            # per-layer weight tiles so step 0 only waits for layer 0
            wsbs = []
            for i in range(NUMFF):
                w_i = cpool.tile([128, KC * D], W_DT, name=f"wsb{i}", tag=f"wsb{i}")
                nc.sync.dma_start(
                    out=w_i[:, :], in_=wff[:, i * KC * D : (i + 1) * KC * D]
                )
                wsbs.append(w_i)
            embsb = cpool.tile([128, T * BL], BF16, tag="embsb")
            nc.sync.dma_start(out=embsb[:, :], in_=emb[:, :])
            headsb = cpool.tile([128, VOCAB], W_DT, tag="headsb")
            nc.sync.dma_start(out=headsb[:, :], in_=headw[:, :])
            onehsb = cpool.tile([128, ntile * VOCAB], F32, tag="onehsb")
            nc.sync.dma_start(out=onehsb[:, :], in_=oneh[:, :])
            wexpsb = cpool.tile([128, 4 * VOCAB], F32, tag="wexpsb")
            nc.sync.dma_start(out=wexpsb[:, :], in_=wexp[:, :])

            # ---- persistent state ----
            readst = cpool.tile([128, T * BL], BF16, tag="readst")
            sumexp = cpool.tile([128, ntile], F32, tag="sumexp")
            tokl = cpool.tile([128, max(1, ntile // 4)], F32, tag="tokl")
            # 4 cycling PSUM tiles (one bank each) so DVE activation reads
            # never share a bank with in-flight PE writes (reuse distance
            # = 4 accumulation groups)
            NPS = 4
            psts = [
                pstpool.tile([128, MC * BL // NPS], F32, name=f"pst{j}", tag=f"pst{j}")
                for j in range(NPS)
            ]

            # ---- recurrence ----
            # Matmul emission order per layer: groups 0 and 1 emit k=0..6
            # first (14 pairs independent of the previous layer's final
            # activation chunk), then their k=7 finishers, then groups 2..7.
            # This covers the previous layer's last-chunk activation latency.
            mm_order = (
                [(0, k) for k in range(KC - 1)]
                + [(1, k) for k in range(KC - 1)]
                + [(0, KC - 1), (1, KC - 1)]
            )
            for m in range(2, MC):
                mm_order += [(m, k) for k in range(KC)]

            def emit_act(nc, i, m, src_ps, pc, dst):
                nc.vector.scalar_tensor_tensor(
                    out=dst,
                    in0=src_ps[:, pc : pc + BL],
                    scalar=0.0,
                    in1=thrsb[:, i * MC * BL + m * BL : i * MC * BL + (m + 1) * BL],
                    op0=ALU.add,
                    op1=ALU.is_ge,
                )

            for t in range(T):
                src = xb
                for i in range(NUMFF):
                    last = i == NUMFF - 1
                    wsb = wsbs[i]
                    h = None if last else hpool.tile([128, MC * BL], BF16, tag="h")

                    def dst_for(m):
                        if last:
                            if m == MC - 1:
                                return readst[:, t * BL : (t + 1) * BL]
                            return xb[:, m * BL : (m + 1) * BL]
                        return h[:, m * BL : (m + 1) * BL]

                    for m, k in mm_order:
                        ps = psts[m % NPS]
                        pc = (m // NPS) * BL
                        lo = m * 128
                        nc.tensor.matmul(
                            ps[:, pc : pc + BL],
                            wsb[:, k * D + lo : k * D + lo + 128],
                            src[:, k * BL : (k + 1) * BL],
                            start=(k == 0),
                            stop=(k == KC - 1),
                        )
                        if k == KC - 1:
                            # group m complete: per-chunk activation
                            # h'[m] = (pre' >= thr2) in {1,0}
                            emit_act(nc, i, m, ps, pc, dst_for(m))
                    if i == 0:
                        # refill the embed chunk of x for the NEXT step as
                        # soon as this step's layer-0 matmuls consumed it
                        nc.vector.tensor_copy(
                            xb[:, CARRY // 16 : MC * BL],
                            embsb[:, t * BL : (t + 1) * BL],
                        )
                    if not last:
                        src = h

            # ---- deferred head + loss (4 tiles batched per PSUM bank) ----
            assert ntile % 4 == 0 or ntile < 4
            nj = max(1, ntile // 4)
            per = min(4, ntile)
            for j in range(nj):
                ps2 = ps2pool.tile([128, per * VOCAB], F32, tag="ps2")
                for u in range(per):
                    nc.tensor.matmul(
                        ps2[:, u * VOCAB : (u + 1) * VOCAB],
                        readst[:, (j * per + u) * 128 : (j * per + u + 1) * 128],
                        headsb[:, :],
                        start=True,
                        stop=True,
                    )
                # logits = (2*ps2 - colsum(H)) / 16 ; exp(logits) =
                # exp(ps2/8) * wexp  with wexp = exp(-colsum(H)/16)
                expt = wpool.tile([128, per * VOCAB], F32, tag="expt")
                nc.scalar.activation(
                    expt[:, :],
                    ps2[:, :],
                    AT.Exp,
                    scale=2.0 * LOGIT_SCALE,
                )
                junk2 = wpool.tile([128, per * VOCAB], F32, tag="junk2")
                for u in range(per):
                    nc.vector.scalar_tensor_tensor(
                        out=junk2[:, u * VOCAB : (u + 1) * VOCAB],
                        in0=expt[:, u * VOCAB : (u + 1) * VOCAB],
                        scalar=1.0,
                        in1=wexpsb[:, 0:VOCAB],
                        op0=ALU.mult,
                        op1=ALU.mult,
                        accum_out=sumexp[:, j * per + u : j * per + u + 1],
                    )
                # device part of logit_tok: (2/16) * ps2_tok (host adds the
                # -colsum(H)[tok]/16 correction)
                junk = wpool.tile([128, per * VOCAB], F32, tag="junk")
                nc.vector.scalar_tensor_tensor(
                    out=junk[:, :],
                    in0=ps2[:, :],
                    scalar=2.0 * LOGIT_SCALE,
                    in1=onehsb[:, j * per * VOCAB : (j + 1) * per * VOCAB],
                    op0=ALU.mult,
                    op1=ALU.mult,
                    accum_out=tokl[:, j : j + 1],
                )

            lse = cpool.tile([128, ntile], F32, tag="lse")
            nc.scalar.activation(lse[:, :], sumexp[:, :], AT.Ln)
            r1 = cpool.tile([128, 1], F32, tag="r1")
            r2 = cpool.tile([128, 1], F32, tag="r2")
            po = cpool.tile([128, 1], F32, tag="po")
            nc.vector.tensor_reduce(
                r1[:, :], lse[:, :], axis=mybir.AxisListType.X, op=ALU.add
            )
            nc.vector.tensor_reduce(
                r2[:, :], tokl[:, :], axis=mybir.AxisListType.X, op=ALU.add
            )
            nc.vector.scalar_tensor_tensor(
                out=po[:, :],
                in0=r1[:, :],
                scalar=0.0,
                in1=r2[:, :],
                op0=ALU.add,
                op1=ALU.subtract,
            )
            nc.sync.dma_start(out=res[:, :], in_=po[:, :])

    return nc


def _sgn(a):
    return np.where(np.asarray(a, np.float32) >= 0, 1.0, -1.0).astype(np.float32)


def prepare_inputs(tokens, initial_lat, embed_lat, ff_lat, head_lat, ff_thresh_lat):
    """Host-side transform of the full problem inputs to per-core DRAM maps.

    Activations are sent in {0,1} encoding h' = (h+1)/2, with thresholds
    folded:  pre >= thr  <=>  h'@W >= (thr + colsum(W))/2.
    Returns (in_maps, B, T, host_corr) where host_corr is the token-logit
    correction  sum_{b,t} colsum(H)[tok]/16  to add to the loss sum.
    """
    tokens = np.asarray(tokens).astype(np.int64)
    B, T = tokens.shape
    E = _sgn(embed_lat)                      # [V, 128]
    W = _sgn(ff_lat)                         # [6, 1024, 1024]
    H = _sgn(head_lat)                       # [128, V]
    x0v = _sgn(initial_lat)                  # [1024]
    R = np.round(np.asarray(ff_thresh_lat, np.float64)).astype(np.float32)  # [6,1024]

    # weights: wff[p, (i*KC+k)*D + mcol] = W[i, k*128+p, mcol]
    wff = (
        W.reshape(NUMFF, KC, 128, D).transpose(2, 0, 1, 3).reshape(128, NUMFF * KC * D)
    ).astype(W_NP)
    headw = H.astype(W_NP)                   # [128 r, V]

    # x0 in {0,1}: col = chunk*8 + b, value (x0v+1)/2 replicated over b
    x0t = ((x0v + 1.0) / 2.0).reshape(MC, 128).T     # [p, chunk]
    x0arr = np.repeat(x0t[:, :, None], BL, axis=2).reshape(128, MC * BL).astype(NP_BF16)

    # folded threshold thr2 = (thr + colsum(W))/2, expanded col = i*64+m*8+b
    S = W.sum(axis=1)                        # [6, 1024] colsums
    thr2 = (R + S) / 2.0
    thr2 = thr2.reshape(NUMFF, MC, 128).transpose(2, 0, 1)[:, :, :, None]
    thr2 = np.broadcast_to(thr2, (128, NUMFF, MC, BL)).reshape(128, NUMFF * MC * BL)
    thr2 = np.ascontiguousarray(thr2, np.float32)

    # head colsum corrections
    csH = H.sum(axis=0)                      # [V]
    wexp = np.exp(-csH / 16.0).astype(np.float32)
    wexp = np.ascontiguousarray(np.tile(wexp[None, :], (128, 4)))
    host_corr = float(csH[tokens].sum()) / 16.0

    ntile = T * BL // 128
    in_maps = []
    for c in range(NCORES):
        tc_ = tokens[c * BL : (c + 1) * BL]  # [8, T]
        # emb in {0,1}: emb[p, t*8+b] = (E[tok[b,t], p]+1)/2
        embc = ((E[tc_] + 1.0) / 2.0).transpose(2, 1, 0).reshape(128, T * BL)
        embc = embc.astype(NP_BF16)
        # one-hot over vocab per (t,b) sample, tiled [128 samples x 128 vocab]
        flat = tc_.T.reshape(-1)             # tb = t*8+b
        onehc = (flat[:, None] == np.arange(VOCAB)[None, :]).astype(np.float32)
        onehc = (
            onehc.reshape(ntile, 128, VOCAB).transpose(1, 0, 2).reshape(128, ntile * VOCAB)
        )
        onehc = np.ascontiguousarray(onehc)
        in_maps.append(
            {
                "wff": wff,
                "emb": embc,
                "x0": x0arr,
                "thr": thr2,
                "headw": headw,
                "oneh": onehc,
                "wexp": wexp,
            }
        )
    return in_maps, B, T, host_corr


def _install_axon_trace_hook():
    """The image's antenv lacks axon_hooks; recreate the NTFF profile hook
    via ctypes against libaxon_pjrt.so (mirrors trn_agent_boot.trn_boot)."""
    import contextlib
    import ctypes
    import types

    try:
        from antenv.axon_hooks import get_axon_ntff_profile_hook  # noqa: F401

        return
    except ImportError:
        pass
    so_path = "/opt/axon/libaxon_pjrt.so"
    lib = ctypes.CDLL(so_path)
    if not hasattr(lib, "axon_start_nrt_profile"):
        return
    lib.axon_start_nrt_profile.argtypes = [
        ctypes.POINTER(ctypes.c_int64),
        ctypes.c_size_t,
    ]
    lib.axon_start_nrt_profile.restype = ctypes.c_int64
    lib.axon_stop_nrt_profile.argtypes = [ctypes.c_char_p]
    lib.axon_stop_nrt_profile.restype = ctypes.c_int64

    @contextlib.contextmanager
    def _hook(output_dir, device_ids):
        import jax

        jax.devices()
        if device_ids:
            ids = (ctypes.c_int64 * len(device_ids))(*device_ids)
            rc = lib.axon_start_nrt_profile(ids, len(device_ids))
        else:
            rc = lib.axon_start_nrt_profile(None, 0)
        if rc != 0:
            raise RuntimeError(f"axon_start_nrt_profile rc={rc}")
        try:
            yield
        finally:
            n = lib.axon_stop_nrt_profile(str(output_dir).encode())
            print(f"profile: {n} file(s) written to {output_dir}", file=sys.stderr)

    import antenv

    mod = types.ModuleType("antenv.axon_hooks")
    mod.get_axon_ntff_profile_hook = lambda: _hook
    sys.modules["antenv.axon_hooks"] = mod
    antenv.axon_hooks = mod

    from concourse import bass_utils as bu

    bu.upload_artifacts = lambda tmpdir: f"local://{tmpdir}"


def run(trace=False, tmpdir=None, **inputs):
    in_maps, B, T, host_corr = prepare_inputs(**inputs)
    nc = build_nc(T)
    if not nc.is_finalized():
        nc.finalize()
    if trace:
        _install_axon_trace_hook()
    out = run_bass_kernel_spmd(
        nc, in_maps, core_ids=list(range(NCORES)), trace=trace, tmpdir=tmpdir
    )
    total = host_corr
    for r in out.results:
        total += np.asarray(r["res"], np.float64).sum()
    loss = np.float32(total / (B * T))
    return np.asarray(loss, dtype=np.float32), out


def kernel(**inputs):
    loss, _ = run(trace=False, **inputs)
    return loss


if __name__ == "__main__":
    # tiny smoke test
    import jax

    sys.path.insert(0, "/root/problem")
    import reference

    inputs = reference.setup_inputs()
    inputs = {k: np.asarray(v) for k, v in inputs.items()}
    Tsmall = int(sys.argv[1]) if len(sys.argv) > 1 else 16
    inputs["tokens"] = inputs["tokens"][:, :Tsmall]
    expected = np.asarray(reference.reference(**{k: v for k, v in inputs.items()}))
    got = kernel(**inputs)
    rel = abs(float(got) - float(expected)) / max(1e-12, abs(float(expected)))
    print(f"T={Tsmall} expected={expected} got={got} rel_err={rel:.3e}")



# revision 4
# speedup vs baseline: 1.0359x; 1.0359x over previous
"""Trainium2 Bass kernel for nn_BRNN_8151847927833.

Binary RNN: B=64 seqs, T=512 steps, d_model=1024, 6 binary FF layers per
step, then a small head + log_softmax + NLL loss averaged over (t, b).

Strategy (data-parallel over batch, 8 cores x 8 sequences):
  - All weights are +-1 (sign of latents), thresholds are small integers.
    Matmuls are therefore EXACT in low precision: products are +-1 and
    PSUM accumulates in fp32.
  - Activations kept transposed ([128 partitions, chunk*8+b]) in a {0,1}
    fp8 encoding (h' = (h+1)/2) with host-folded thresholds
    (thr + colsum(W))/2, so each layer's nonlinearity is an exact DVE
    is_ge compare.
  - PE work: per layer 8x8 (k,m) weight-stationary pairs (LDWEIGHTS +
    MATMUL, fp8 so FWL is active).  Emission is split into block A
    (k=0..3, gated only on the previous layer's low-half activation) and
    block B (k=4..7, gated on the high half), so the DVE round-trip
    latency of each half-activation hides under ~16-32 matmul pairs.
  - Activations are 2 batched DVE ops per layer ([128,32] halves reading
    separate PSUM banks; PE writes one bank while DVE reads the other),
    instead of 8 tiny [128,8] ops -- DVE busy drops ~4x and the PE is
    never gated on a chain of small DVE ops.
  - Layer 5 (last) only computes the 896 carry dims (7 groups) in-loop.
    Its input h5 is written straight into a [128, T, 64] SBUF ring; the
    read chunk (dims 896..1023) AND the head/log-softmax/token-gather are
    recomputed from h5 after the T-loop as dense batched matmuls
    (N=128 samples per matmul).
  - The embed chunk of step t+1's input is copied on the GpSimd engine
    (SBUF->SBUF) at the start of step t, off the DVE/PE critical path.
  - Each core returns per-partition partial sums of (logsumexp -
    logit_tok); the host sums across cores and divides by B*T.
"""

import math
import sys

import numpy as np

sys.path.insert(0, "/opt/trn_rl_repo")

import ml_dtypes  # noqa: E402

import concourse.bass as bass  # noqa: E402
import concourse.bacc as bacc  # noqa: E402
import concourse.mybir as mybir  # noqa: E402
from concourse.tile import TileContext  # noqa: E402
from concourse.bass_utils import run_bass_kernel_spmd  # noqa: E402

F32 = mybir.dt.float32
BF16 = mybir.dt.bfloat16
FP8 = mybir.dt.float8e4
NP_BF16 = ml_dtypes.bfloat16
NP_FP8 = ml_dtypes.float8_e4m3

D = 1024          # d_model
KC = 8            # contraction chunks of 128
MC = 8            # output chunks of 128
NUMFF = 6
VOCAB = 128
READ = 128
CARRY = 896
BL = 8            # batch per core
NCORES = 8
LOGIT_SCALE = 1.0 / 16.0

W_DT = FP8
W_NP = NP_FP8


def build_nc(T):
    """Build the SPMD Bass kernel for T timesteps (BL sequences/core)."""
    ntile = T * BL // 128  # sample tiles over (t, b)
    assert T * BL % 128 == 0
    steps_per_tile = 128 // BL  # 16

    nc = bacc.Bacc("TRN2", target_bir_lowering=False)
    wff = nc.dram_tensor("wff", [128, NUMFF * KC * D], W_DT, kind="ExternalInput")
    emb = nc.dram_tensor("emb", [128, T * BL], W_DT, kind="ExternalInput")
    x0 = nc.dram_tensor("x0", [128, MC * BL], W_DT, kind="ExternalInput")
    thr = nc.dram_tensor("thr", [128, NUMFF * MC * BL], F32, kind="ExternalInput")
    w6r = nc.dram_tensor("w6r", [128, KC * READ], W_DT, kind="ExternalInput")
    thrr = nc.dram_tensor("thrr", [128, 1], F32, kind="ExternalInput")
    headw = nc.dram_tensor("headw", [128, VOCAB], W_DT, kind="ExternalInput")
    oneh = nc.dram_tensor("oneh", [128, ntile * VOCAB], W_DT, kind="ExternalInput")
    wexp = nc.dram_tensor("wexp", [128, 4 * VOCAB], F32, kind="ExternalInput")
    res = nc.dram_tensor("res", [128, 1], F32, kind="ExternalOutput")

    AT = mybir.ActivationFunctionType
    ALU = mybir.AluOpType

    with TileContext(nc) as tc:
        with (
            tc.tile_pool(name="const", bufs=1) as cpool,
            tc.tile_pool(name="work", bufs=4) as wpool,
            tc.tile_pool(name="hpool", bufs=3) as hpool,
        ):
            # ---- resident inputs (DMA in consumption order) ----
            xb = cpool.tile([128, MC * BL], W_DT, tag="xb")
            nc.sync.dma_start(out=xb[:, :], in_=x0[:, :])
            thrsb = cpool.tile([128, NUMFF * MC * BL], F32, tag="thrsb")
            nc.sync.dma_start(out=thrsb[:, :], in_=thr[:, :])
            # per-layer weight tiles so step 0 only waits for layer 0
            wsbs = []
            for i in range(NUMFF):
                w_i = cpool.tile([128, KC, D], W_DT, name=f"wsb{i}", tag=f"wsb{i}")
                nc.sync.dma_start(
                    out=w_i[:, :, :], in_=wff[:, i * KC * D : (i + 1) * KC * D]
                )
                wsbs.append(w_i)
            embsb = cpool.tile([128, T * BL], W_DT, tag="embsb")
            nc.sync.dma_start(out=embsb[:, :], in_=emb[:, :])
            w6rsb = cpool.tile([128, KC * READ], W_DT, tag="w6rsb")
            nc.sync.dma_start(out=w6rsb[:, :], in_=w6r[:, :])
            thrrsb = cpool.tile([128, 1], F32, tag="thrrsb")
            nc.sync.dma_start(out=thrrsb[:, :], in_=thrr[:, :])
            headsb = cpool.tile([128, VOCAB], W_DT, tag="headsb")
            nc.sync.dma_start(out=headsb[:, :], in_=headw[:, :])
            onehsb = cpool.tile([128, ntile * VOCAB], W_DT, tag="onehsb")
            nc.sync.dma_start(out=onehsb[:, :], in_=oneh[:, :])
            wexpsb = cpool.tile([128, 4 * VOCAB], F32, tag="wexpsb")
            nc.sync.dma_start(out=wexpsb[:, :], in_=wexp[:, :])

            # ---- persistent state ----
            h5buf = cpool.tile([128, T, MC * BL], W_DT, tag="h5buf")
            sumexp = cpool.tile([128, ntile], F32, tag="sumexp")
            tokl = cpool.tile([128, max(1, ntile // 4)], F32, tag="tokl")

            # ---- recurrence ----
            # Emission order per layer: 32 units of (m-group, k-pair), 2
            # matmuls each.  PSUM start=True zeroes has_written bits for the
            # WHOLE bank, so the two groups sharing a bank must accumulate
            # strictly sequentially (group 2j fully, then group 2j+1).  The
            # order interleaves banks so that (a) early units only need
            # early-arriving activations of the previous layer (k01/k23
            # first, k45 from ~unit 6, k67 from ~unit 9), and (b) bank j's
            # two groups finish progressively earlier for smaller j, so
            # act_j of this layer lands before the next layer needs chunk
            # pair j.  (m, c): k = 2c, 2c+1.
            UNITS = [
                (0, 0), (0, 1), (2, 0), (2, 1), (4, 0), (0, 2), (4, 1),
                (6, 0), (0, 3),
                (1, 0), (1, 1), (2, 2), (1, 2), (6, 1), (1, 3),
                (2, 3),
                (3, 0), (3, 1), (4, 2), (3, 2), (3, 3),
                (4, 3),
                (5, 0), (5, 1), (6, 2), (5, 2), (5, 3),
                (6, 3),
                (7, 0), (7, 1), (7, 2), (7, 3),
            ]
            # sanity: per-group c ascending; per-bank sequential groups
            for m in range(MC):
                cs = [c for (mm, c) in UNITS if mm == m]
                assert cs == [0, 1, 2, 3]
            for j in range(4):
                stop_idx = UNITS.index((2 * j, 3))
                start_idx = UNITS.index((2 * j + 1, 0))
                assert stop_idx < start_idx
            # act_j emitted right after the last unit of its bank
            act_after = {}
            for j in range(4):
                act_after[UNITS.index((2 * j + 1, 3))] = j
            act_after_l5 = {}
            for j in range(3):
                act_after_l5[UNITS.index((2 * j + 1, 3))] = j
            act_after_l5[UNITS.index((6, 3))] = 3
            UNITS_L5 = [u for u in UNITS if u[0] != 7]

            with tc.tile_pool(name="psq", bufs=2, space="PSUM") as psq:
                x_cur = xb
                for t in range(T):
                    x_next = hpool.tile([128, MC * BL], W_DT, tag="xn")
                    # embed chunk of step t+1's input: fill early, off the
                    # critical path (GpSimd, SBUF->SBUF)
                    nc.gpsimd.tensor_copy(
                        x_next[:, CARRY // 16 : MC * BL],
                        embsb[:, t * BL : (t + 1) * BL],
                    )
                    for i in range(NUMFF):
                        last = i == NUMFF - 1
                        w3 = wsbs[i]
                        if i == 4:
                            dst = h5buf[:, t, :]
                        elif last:
                            dst = x_next
                        else:
                            dst = hpool.tile([128, MC * BL], W_DT, tag="h")
                        src = h5buf[:, t, :] if last else x_cur

                        pbs = [
                            psq.tile([128, 16], F32, name=f"pb{j}", tag=f"pb{j}")
                            for j in range(4)
                        ]
                        units = UNITS_L5 if last else UNITS
                        acts = act_after_l5 if last else act_after

                        def emit_act(j):
                            w = 8 if (last and j == 3) else 16
                            nc.vector.scalar_tensor_tensor(
                                out=dst[:, j * 16 : j * 16 + w],
                                in0=pbs[j][:, 0:w],
                                scalar=0.0,
                                in1=thrsb[:, i * MC * BL + j * 16 : i * MC * BL + j * 16 + w],
                                op0=ALU.add,
                                op1=ALU.is_ge,
                            )

                        for ui, (m, c) in enumerate(units):
                            ps = pbs[m // 2]
                            mo = m % 2
                            for k in (2 * c, 2 * c + 1):
                                nc.tensor.matmul(
                                    ps[:, mo * BL : (mo + 1) * BL],
                                    w3[:, k, m * 128 : (m + 1) * 128],
                                    src[:, k * BL : (k + 1) * BL],
                                    start=(k == 0),
                                    stop=(k == KC - 1),
                                )
                            if ui in acts:
                                emit_act(acts[ui])
                        if i < 4:
                            x_cur = dst
                    x_cur = x_next

            # ---- deferred read chunk + head + loss ----
            assert ntile % 4 == 0 or ntile < 4
            nj = max(1, ntile // 4)
            per = min(4, ntile)
            with (
                tc.tile_pool(name="psr", bufs=2, space="PSUM") as psrpool,
                tc.tile_pool(name="ps2", bufs=2, space="PSUM") as ps2pool,
            ):
                for j in range(nj):
                    ps2 = ps2pool.tile([128, per * VOCAB], F32, tag="ps2")
                    readt = wpool.tile([128, per * 128], W_DT, tag="readt")
                    for u in range(per):
                        t0 = (j * per + u) * steps_per_tile
                        # read chunk: [128 rdim, 128 samples] over 8 k-chunks
                        psr = psrpool.tile([128, 128], F32, tag="psr")
                        for k in range(KC):
                            nc.tensor.matmul(
                                psr[:, :],
                                w6rsb[:, k * READ : (k + 1) * READ],
                                h5buf[:, t0 : t0 + steps_per_tile, k * BL : (k + 1) * BL],
                                start=(k == 0),
                                stop=(k == KC - 1),
                            )
                        nc.vector.tensor_scalar(
                            out=readt[:, u * 128 : (u + 1) * 128],
                            in0=psr[:, :],
                            scalar1=thrrsb[:, 0:1],
                            scalar2=None,
                            op0=ALU.is_ge,
                        )
                        nc.tensor.matmul(
                            ps2[:, u * VOCAB : (u + 1) * VOCAB],
                            readt[:, u * 128 : (u + 1) * 128],
                            headsb[:, :],
                            start=True,
                            stop=True,
                        )
                    # logits = (2*ps2 - colsum(H)) / 16 ; exp(logits) =
                    # exp(ps2/8) * wexp  with wexp = exp(-colsum(H)/16)
                    expt = wpool.tile([128, per * VOCAB], F32, tag="expt")
                    nc.scalar.activation(
                        expt[:, :],
                        ps2[:, :],
                        AT.Exp,
                        scale=2.0 * LOGIT_SCALE,
                    )
                    junk2 = wpool.tile([128, per * VOCAB], F32, tag="junk2")
                    for u in range(per):
                        nc.vector.scalar_tensor_tensor(
                            out=junk2[:, u * VOCAB : (u + 1) * VOCAB],
                            in0=expt[:, u * VOCAB : (u + 1) * VOCAB],
                            scalar=1.0,
                            in1=wexpsb[:, 0:VOCAB],
                            op0=ALU.mult,
                            op1=ALU.mult,
                            accum_out=sumexp[:, j * per + u : j * per + u + 1],
                        )
                    # device part of logit_tok: (2/16) * ps2_tok (host adds
                    # the -colsum(H)[tok]/16 correction)
                    junk = wpool.tile([128, per * VOCAB], F32, tag="junk")
                    nc.vector.scalar_tensor_tensor(
                        out=junk[:, :],
                        in0=ps2[:, :],
                        scalar=2.0 * LOGIT_SCALE,
                        in1=onehsb[:, j * per * VOCAB : (j + 1) * per * VOCAB],
                        op0=ALU.mult,
                        op1=ALU.mult,
                        accum_out=tokl[:, j : j + 1],
                    )

                lse = cpool.tile([128, ntile], F32, tag="lse")
                nc.scalar.activation(lse[:, :], sumexp[:, :], AT.Ln)
                r1 = cpool.tile([128, 1], F32, tag="r1")
                r2 = cpool.tile([128, 1], F32, tag="r2")
                po = cpool.tile([128, 1], F32, tag="po")
                nc.vector.tensor_reduce(
                    r1[:, :], lse[:, :], axis=mybir.AxisListType.X, op=ALU.add
                )
                nc.vector.tensor_reduce(
                    r2[:, :], tokl[:, :], axis=mybir.AxisListType.X, op=ALU.add
                )
                nc.vector.scalar_tensor_tensor(
                    out=po[:, :],
                    in0=r1[:, :],
                    scalar=0.0,
                    in1=r2[:, :],
                    op0=ALU.add,
                    op1=ALU.subtract,
                )
                nc.sync.dma_start(out=res[:, :], in_=po[:, :])

    return nc


def _sgn(a):
    return np.where(np.asarray(a, np.float32) >= 0, 1.0, -1.0).astype(np.float32)


def prepare_inputs(tokens, initial_lat, embed_lat, ff_lat, head_lat, ff_thresh_lat):
    """Host-side transform of the full problem inputs to per-core DRAM maps.

    Activations are sent in {0,1} encoding h' = (h+1)/2, with thresholds
    folded:  pre >= thr  <=>  h'@W >= (thr + colsum(W))/2.
    Returns (in_maps, B, T, host_corr) where host_corr is the token-logit
    correction  sum_{b,t} colsum(H)[tok]/16  to add to the loss sum.
    """
    tokens = np.asarray(tokens).astype(np.int64)
    B, T = tokens.shape
    E = _sgn(embed_lat)                      # [V, 128]
    W = _sgn(ff_lat)                         # [6, 1024, 1024]
    H = _sgn(head_lat)                       # [128, V]
    x0v = _sgn(initial_lat)                  # [1024]
    R = np.round(np.asarray(ff_thresh_lat, np.float64)).astype(np.float32)  # [6,1024]

    # weights: wff[p, (i*KC+k)*D + mcol] = W[i, k*128+p, mcol]
    wff = (
        W.reshape(NUMFF, KC, 128, D).transpose(2, 0, 1, 3).reshape(128, NUMFF * KC * D)
    ).astype(W_NP)
    headw = H.astype(W_NP)                   # [128 r, V]
    # read-chunk weights: w6r[p, k*READ + r] = W[5, k*128+p, CARRY+r]
    w6r = (
        W[5, :, CARRY:].reshape(KC, 128, READ).transpose(1, 0, 2).reshape(128, KC * READ)
    ).astype(W_NP)

    # x0 in {0,1}: col = chunk*8 + b, value (x0v+1)/2 replicated over b
    x0t = ((x0v + 1.0) / 2.0).reshape(MC, 128).T     # [p, chunk]
    x0arr = np.repeat(x0t[:, :, None], BL, axis=2).reshape(128, MC * BL).astype(W_NP)

    # folded threshold thr2 = (thr + colsum(W))/2, expanded col = i*64+m*8+b
    S = W.sum(axis=1)                        # [6, 1024] colsums
    thr2 = (R + S) / 2.0
    thr2f = thr2.reshape(NUMFF, MC, 128).transpose(2, 0, 1)[:, :, :, None]
    thr2f = np.broadcast_to(thr2f, (128, NUMFF, MC, BL)).reshape(128, NUMFF * MC * BL)
    thr2f = np.ascontiguousarray(thr2f, np.float32)
    # read-dim thresholds, per partition
    thrr = np.ascontiguousarray(thr2[5, CARRY:].reshape(128, 1), np.float32)

    # head colsum corrections
    csH = H.sum(axis=0)                      # [V]
    wexp = np.exp(-csH / 16.0).astype(np.float32)
    wexp = np.ascontiguousarray(np.tile(wexp[None, :], (128, 4)))
    host_corr = float(csH[tokens].sum()) / 16.0

    ntile = T * BL // 128
    in_maps = []
    for c in range(NCORES):
        tc_ = tokens[c * BL : (c + 1) * BL]  # [8, T]
        # emb in {0,1}: emb[p, t*8+b] = (E[tok[b,t], p]+1)/2
        embc = ((E[tc_] + 1.0) / 2.0).transpose(2, 1, 0).reshape(128, T * BL)
        embc = embc.astype(W_NP)
        # one-hot over vocab per (t,b) sample, tiled [128 samples x 128 vocab]
        flat = tc_.T.reshape(-1)             # tb = t*8+b
        onehc = (flat[:, None] == np.arange(VOCAB)[None, :]).astype(np.float32)
        onehc = (
            onehc.reshape(ntile, 128, VOCAB).transpose(1, 0, 2).reshape(128, ntile * VOCAB)
        )
        onehc = np.ascontiguousarray(onehc).astype(W_NP)
        in_maps.append(
            {
                "wff": wff,
                "emb": embc,
                "x0": x0arr,
                "thr": thr2f,
                "w6r": w6r,
                "thrr": thrr,
                "headw": headw,
                "oneh": onehc,
                "wexp": wexp,
            }
        )
    return in_maps, B, T, host_corr


def _install_axon_trace_hook():
    """The image's antenv lacks axon_hooks; recreate the NTFF profile hook
    via ctypes against libaxon_pjrt.so (mirrors trn_agent_boot.trn_boot)."""
    import contextlib
    import ctypes
    import types

    try:
        from antenv.axon_hooks import get_axon_ntff_profile_hook  # noqa: F401

        return
    except ImportError:
        pass
    so_path = "/opt/axon/libaxon_pjrt.so"
    lib = ctypes.CDLL(so_path)
    if not hasattr(lib, "axon_start_nrt_profile"):
        return
    lib.axon_start_nrt_profile.argtypes = [
        ctypes.POINTER(ctypes.c_int64),
        ctypes.c_size_t,
    ]
    lib.axon_start_nrt_profile.restype = ctypes.c_int64
    lib.axon_stop_nrt_profile.argtypes = [ctypes.c_char_p]
    lib.axon_stop_nrt_profile.restype = ctypes.c_int64

    @contextlib.contextmanager
    def _hook(output_dir, device_ids):
        import jax

        jax.devices()
        if device_ids:
            ids = (ctypes.c_int64 * len(device_ids))(*device_ids)
            rc = lib.axon_start_nrt_profile(ids, len(device_ids))
        else:
            rc = lib.axon_start_nrt_profile(None, 0)
        if rc != 0:
            raise RuntimeError(f"axon_start_nrt_profile rc={rc}")
        try:
            yield
        finally:
            n = lib.axon_stop_nrt_profile(str(output_dir).encode())
            print(f"profile: {n} file(s) written to {output_dir}", file=sys.stderr)

    import antenv

    mod = types.ModuleType("antenv.axon_hooks")
    mod.get_axon_ntff_profile_hook = lambda: _hook
    sys.modules["antenv.axon_hooks"] = mod
    antenv.axon_hooks = mod

    from concourse import bass_utils as bu

    bu.upload_artifacts = lambda tmpdir: f"local://{tmpdir}"


def run(trace=False, tmpdir=None, **inputs):
    in_maps, B, T, host_corr = prepare_inputs(**inputs)
    nc = build_nc(T)
    if not nc.is_finalized():
        nc.finalize()
    if trace:
        _install_axon_trace_hook()
    out = run_bass_kernel_spmd(
        nc, in_maps, core_ids=list(range(NCORES)), trace=trace, tmpdir=tmpdir
    )
    total = host_corr
    for r in out.results:
        total += np.asarray(r["res"], np.float64).sum()
    loss = np.float32(total / (B * T))
    return np.asarray(loss, dtype=np.float32), out


def kernel(**inputs):
    loss, _ = run(trace=False, **inputs)
    return loss


if __name__ == "__main__":
    # tiny smoke test
    import jax

    sys.path.insert(0, "/root/problem")
    import reference

    inputs = reference.setup_inputs()
    inputs = {k: np.asarray(v) for k, v in inputs.items()}
    Tsmall = int(sys.argv[1]) if len(sys.argv) > 1 else 16
    inputs["tokens"] = inputs["tokens"][:, :Tsmall]
    expected = np.asarray(reference.reference(**{k: v for k, v in inputs.items()}))
    got = kernel(**inputs)
    rel = abs(float(got) - float(expected)) / max(1e-12, abs(float(expected)))
    print(f"T={Tsmall} expected={expected} got={got} rel_err={rel:.3e}")
